# revision 14
# baseline (speedup 1.0000x reference)
"""DynamicConv2D Trainium2 kernel (8-core SPMD, data-parallel over batch).

Per sample: GAP -> MLP -> softmax routing over K=4 kernel banks, weight-space
aggregation, then a 3x3 SAME conv with the per-sample aggregated kernel.

Device strategy (per core, 4 samples, software-pipelined across samples):
  - Host packs x TRANSPOSED, width-padded, channel-duplicated bf16
    [128, SP] per sample (rows 0:64 = channels at spatial s, rows 64:128 =
    same shifted one padded image row, so row 64+c at col s holds
    x[c, s-WP]). Plain chunked DMAs load it -- no xbar transposes.
  - Engine/queue assignment keeps every FIFO conflict-free:
      SP    : input chunk loads only
      ACT   : PSUM drains (+bias) and the output store only
      DVE   : pooled partials + the whole routing chain (softmax via a
              3rd-order Taylor exp -- |logits/T| < 1e-3 -- so no ACT hop)
      GpSimd: pad memsets, one pooled partial, pi partition-broadcast
      PE    : routing MLP matmuls + conv matmuls
  - Software pipelining: per iteration emit loads+partials for sample b,
    conv second half of b-1, routing chain of b, conv first half of b.
    The tiny MLP matmuls land mid-stream in the PE queue so the PE never
    idles long enough to drop out of HAM warm state.
  - Conv as shifted matmuls accumulating in PSUM: out[f, p] tiles, f on
    partitions; K=128 paired taps (dy=0 with dy=-1) + K=64 single taps
    (dy=+1); col groups 0/64 of the PE run image halves A/B concurrently.
  - One plain DMA stores yT [f | spatial] per sample; host un-transposes,
    strips width pads, upcasts to fp32.
"""

import numpy as np
import ml_dtypes

BF16 = ml_dtypes.bfloat16

B, H, W, C, F = 32, 128, 128, 64, 64
KK, HID = 4, 16
TEMP = 30.0
NCORES, BPC = 8, 4
WP = W + 2          # padded width (zero col at w'=0 and w'=129)
SP = H * WP         # 16640 padded spatial per sample
PADL = 384          # SBUF zero halo before the image
PADR = 384          # SBUF zero halo after (taps read up to +WP+1)
NTMAX = 512         # matmul moving-dim tile (PSUM bank: <=512 fp32)
HALF = SP // 2      # 8320, image halves A (h<64) / B (h>=64)
TILES = [(i * NTMAX, NTMAX) for i in range(HALF // NTMAX)]
if HALF % NTMAX:
    TILES.append((HALF - HALF % NTMAX, HALF % NTMAX))
TPH = len(TILES)    # 17 tiles per half (16x512 + 1x128)
NSLOT = 6           # 3 paired-tap slots (K=128) + 3 single-tap slots (K=64)
CHW = SP // 4       # input DMA chunk width (4160 cols, ~1.06 MB)

_CACHE = {}


def _build_program(dbg=False, reps=1):
    import concourse.bacc as bacc
    import concourse.mybir as mybir
    import concourse.tile as tile

    f32 = mybir.dt.float32
    bf16 = mybir.dt.bfloat16
    AX = mybir.AxisListType.X
    ALU = mybir.AluOpType

    nc = bacc.Bacc("TRN2", target_bir_lowering=False, debug=False)

    x2_d = nc.dram_tensor("x2", [BPC, 128, SP], bf16, kind="ExternalInput")
    wk_d = nc.dram_tensor("wk", [128, KK * NSLOT * F], f32,
                          kind="ExternalInput")
    w1_d = nc.dram_tensor("w1", [C, HID], f32, kind="ExternalInput")
    b1_d = nc.dram_tensor("b1", [HID, 1], f32, kind="ExternalInput")
    w2_d = nc.dram_tensor("w2", [HID, KK], f32, kind="ExternalInput")
    b2_d = nc.dram_tensor("b2", [1, KK], f32, kind="ExternalInput")
    bkt_d = nc.dram_tensor("bkt", [128, KK], f32, kind="ExternalInput")
    yp_d = nc.dram_tensor("ypad", [BPC, 128, HALF], bf16,
                          kind="ExternalOutput")
    if dbg:
        dpool_d = nc.dram_tensor("dpool", [BPC, C, 1], f32,
                                 kind="ExternalOutput")
        dpib_d = nc.dram_tensor("dpib", [BPC, 128, KK], f32,
                                kind="ExternalOutput")
        dwg_d = nc.dram_tensor("dwg", [BPC, 128, NSLOT * F], bf16,
                               kind="ExternalOutput")

    with tile.TileContext(nc) as tc:
        from contextlib import ExitStack
        with ExitStack() as ctx:
            cst = ctx.enter_context(tc.tile_pool(name="cst", bufs=1))
            xtp = ctx.enter_context(tc.tile_pool(name="xtp", bufs=3))
            ytp = ctx.enter_context(tc.tile_pool(name="ytp", bufs=2))
            wgp = ctx.enter_context(tc.tile_pool(name="wgp", bufs=2))
            smp = ctx.enter_context(tc.tile_pool(name="smp", bufs=2))
            psp = ctx.enter_context(tc.tile_pool(name="psp", bufs=6,
                                                 space="PSUM"))
            psr = ctx.enter_context(tc.tile_pool(name="psr", bufs=1,
                                                 space="PSUM"))

            # ---- constants ----
            wk_t = cst.tile([128, KK * NSLOT * F], f32)
            nc.sync.dma_start(wk_t[:], wk_d.ap())
            w1_t = cst.tile([C, HID], f32)
            nc.sync.dma_start(w1_t[:], w1_d.ap())
            b1_t = cst.tile([HID, 1], f32)
            nc.sync.dma_start(b1_t[:], b1_d.ap())
            w2_t = cst.tile([HID, KK], f32)
            nc.sync.dma_start(w2_t[:], w2_d.ap())
            b2_t = cst.tile([1, KK], f32)
            nc.sync.dma_start(b2_t[:], b2_d.ap())
            bkt_t = cst.tile([128, KK], f32)
            nc.sync.dma_start(bkt_t[:], bkt_d.ap())
            bagg_t = cst.tile([128, BPC], f32)

            def emit_load(b):
                """Input chunk DMAs + pooled partial reductions."""
                xt = xtp.tile([128, PADL + SP + PADR], bf16, tag="xt")
                nc.gpsimd.memset(xt[:, 0:PADL], 0.0)
                nc.gpsimd.memset(xt[:, PADL + SP:PADL + SP + PADR], 0.0)
                for i in range(4):
                    o = i * CHW
                    nc.sync.dma_start(xt[:, PADL + o:PADL + o + CHW],
                                      x2_d.ap()[b][:, o:o + CHW])
                pp = smp.tile([C, 4], f32, tag="pp")
                for i in range(4):
                    nc.vector.reduce_sum(
                        pp[:, i:i + 1],
                        xt[0:C, PADL + i * CHW:PADL + (i + 1) * CHW],
                        axis=AX)
                return xt, pp

            def emit_chain(b, pp):
                """Routing: pooled -> MLP -> unnormalized softmax -> W_agg.

                The softmax stays UNNORMALIZED (wg = sum_k ex_k Wk); the
                1/sum factor rides into the PSUM drains as a per-partition
                activation scale, so sum/reciprocal leave the critical path.
                """
                pooled = smp.tile([C, 1], f32, tag="pooled")
                nc.vector.reduce_sum(pooled[:], pp[:], axis=AX)
                hps = psr.tile([HID, 1], f32, tag="hps")
                nc.tensor.matmul(hps[:], lhsT=w1_t[:], rhs=pooled[:],
                                 start=True, stop=True)
                h_t = smp.tile([HID, 1], f32, tag="h")
                nc.vector.tensor_scalar(h_t[:], hps[:],
                                        scalar1=b1_t[:, 0:1], scalar2=0.0,
                                        op0=ALU.add, op1=ALU.max)
                lps = psr.tile([1, KK], f32, tag="lps")
                nc.tensor.matmul(lps[:], lhsT=h_t[:], rhs=w2_t[:],
                                 start=True, stop=True)
                lg = smp.tile([1, KK], f32, tag="lg")
                nc.vector.tensor_tensor(lg[:], lps[:], b2_t[:], op=ALU.add)
                # exp(z) ~= 1 + z + z^2/2   (|z| < 1e-3 here)
                e1 = smp.tile([1, KK], f32, tag="e1")
                nc.vector.tensor_scalar(e1[:], lg[:], scalar1=0.5,
                                        scalar2=1.0, op0=ALU.mult,
                                        op1=ALU.add)
                nc.vector.tensor_tensor(e1[:], e1[:], lg[:], op=ALU.mult)
                ex = smp.tile([1, KK], f32, tag="ex")
                nc.vector.tensor_scalar(ex[:], e1[:], scalar1=1.0,
                                        scalar2=None, op0=ALU.add)
                pib = smp.tile([128, KK], f32, tag="pib")
                nc.gpsimd.partition_broadcast(pib[:], ex[:])

                # W_agg = sum_k ex_k * Wk (last FMA writes bf16 directly)
                SF = NSLOT * F
                acc = wgp.tile([128, SF], f32, tag="acc")
                nc.vector.tensor_scalar(acc[:], wk_t[:, 0:SF],
                                        scalar1=pib[:, 0:1], scalar2=None,
                                        op0=ALU.mult)
                for k in range(1, KK - 1):
                    nc.vector.scalar_tensor_tensor(
                        acc[:], wk_t[:, k * SF:(k + 1) * SF],
                        pib[:, k:k + 1], acc[:], op0=ALU.mult, op1=ALU.add)
                wg = wgp.tile([128, SF], bf16, tag="wg")
                nc.vector.scalar_tensor_tensor(
                    wg[:], wk_t[:, (KK - 1) * SF:KK * SF],
                    pib[:, KK - 1:KK], acc[:], op0=ALU.mult, op1=ALU.add)

                # off critical path: 1/sum(ex) for the drains + bias column
                sm = smp.tile([1, 1], f32, tag="sm")
                nc.vector.reduce_sum(sm[:], ex[:], axis=AX)
                rc = smp.tile([1, 1], f32, tag="rc")
                nc.vector.reciprocal(rc[:], sm[:])
                rcb = smp.tile([128, 1], f32, tag="rcb")
                nc.gpsimd.partition_broadcast(rcb[:], rc[:])
                # bagg[:, b] = (sum_k bkT[:, k] * ex_k) * rc
                bu = smp.tile([128, 1], f32, tag="bu")
                nc.vector.tensor_scalar(bu[:], bkt_t[:, 0:1],
                                        scalar1=pib[:, 0:1], scalar2=None,
                                        op0=ALU.mult)
                for k in range(1, KK):
                    nc.vector.scalar_tensor_tensor(
                        bu[:], bkt_t[:, k:k + 1],
                        pib[:, k:k + 1], bu[:],
                        op0=ALU.mult, op1=ALU.add)
                nc.vector.tensor_scalar(bagg_t[:, b:b + 1], bu[:],
                                        scalar1=rcb[:, 0:1], scalar2=None,
                                        op0=ALU.mult)
                if dbg:
                    nc.sync.dma_start(dpool_d.ap()[b], pooled[:])
                    nc.sync.dma_start(dpib_d.ap()[b], pib[:])
                    nc.sync.dma_start(dwg_d.ap()[b], wg[:])
                return wg, rcb

            def emit_conv(b, xt, wg, rcb, yt, t0, t1):
                for t in range(t0, t1):
                    o, w = TILES[t]
                    ps = psp.tile([128, NTMAX], f32, tag="ps")
                    oA = PADL + o
                    oB = oA + HALF
                    for j in range(3):       # taps (0,dx)+(-1,dx), K=128
                        off = j - 1
                        nc.tensor.matmul(
                            ps[0:64, 0:w], lhsT=wg[:, j * F:(j + 1) * F],
                            rhs=xt[:, oA + off:oA + off + w],
                            start=(j == 0), stop=False)
                        nc.tensor.matmul(
                            ps[64:128, 0:w], lhsT=wg[:, j * F:(j + 1) * F],
                            rhs=xt[:, oB + off:oB + off + w],
                            start=(j == 0), stop=False,
                            tile_position=(0, 64))
                    for j in range(3, 6):    # taps (+1,dx), K=64
                        off = WP + (j - 4)
                        nc.tensor.matmul(
                            ps[0:64, 0:w],
                            lhsT=wg[0:64, j * F:(j + 1) * F],
                            rhs=xt[0:64, oA + off:oA + off + w],
                            start=False, stop=(j == 5))
                        nc.tensor.matmul(
                            ps[64:128, 0:w],
                            lhsT=wg[0:64, j * F:(j + 1) * F],
                            rhs=xt[0:64, oB + off:oB + off + w],
                            start=False, stop=(j == 5),
                            tile_position=(0, 64))
                    nc.scalar.activation(
                        yt[:, o:o + w], ps[:, 0:w],
                        mybir.ActivationFunctionType.Identity,
                        bias=bagg_t[:, b:b + 1], scale=rcb[:, 0:1])

            def emit_store(b, yt):
                nc.scalar.dma_start(yp_d.ap()[b], yt[:])

            # ---- software-pipelined main loop: the routing chain for
            # sample s+1 runs during conv of sample s; loads run two ahead.
            S = reps * BPC
            ld = {}
            ch = {}
            ld[0] = emit_load(0)
            ch[0] = emit_chain(0, ld[0][1])
            if S > 1:
                ld[1] = emit_load(1)
            for s in range(S):
                if s + 1 < S:
                    ch[s + 1] = emit_chain((s + 1) % BPC, ld[s + 1][1])
                if s + 2 < S:
                    ld[s + 2] = emit_load((s + 2) % BPC)
                xt, _ = ld.pop(s)
                wg, rcb = ch.pop(s)
                yt = ytp.tile([128, HALF], bf16, tag="yt")
                emit_conv(s % BPC, xt, wg, rcb, yt, 0, TPH)
                emit_store(s % BPC, yt)

    nc.compile()
    return nc


def _get_program():
    if "nc" not in _CACHE:
        _CACHE["nc"] = _build_program()
    return _CACHE["nc"]


def _host_pack_x(x):
    # [B, H, W, C] fp32 -> [B, 128, SP] bf16: rows 0:64 = width-padded x
    # transposed to [c, spatial], rows 64:128 = same, shifted one padded
    # image row (row 64+c at col s holds x[c, s-WP]).
    xb = x.astype(BF16)
    xp = np.zeros((B, H, WP, C), dtype=BF16)
    xp[:, :, 1:W + 1, :] = xb
    flat = xp.reshape(B, SP, C)
    xT = np.ascontiguousarray(flat.transpose(0, 2, 1))   # [B, C, SP]
    x2 = np.zeros((B, 128, SP), dtype=BF16)
    x2[:, 0:C, :] = xT
    x2[:, C:2 * C, WP:] = xT[:, :, 0:SP - WP]
    return x2


def _host_pack_wk(Wk):
    # [K, 3, 3, C, F] -> [128, K*NSLOT*F] fp32. Slot j in 0..2 pairs taps
    # (kh=1, kw=j) on partitions 0:64 with (kh=0, kw=j) on 64:128 (the
    # bottom x half holds the row above). Slots 3/5 hold taps (kh=2,
    # kw=0/2) on partitions 0:64; slot 4 holds tap (kh=2, kw=1) on
    # partitions 64:128 (it reads the shifted copy at offset +2*WP and
    # row-tiles concurrently with slot 3).
    w = np.zeros((128, KK, NSLOT, F), dtype=np.float32)
    wt = np.transpose(Wk, (3, 0, 1, 2, 4))          # [C, K, kh, kw, F]
    for j in range(3):
        w[0:C, :, j] = wt[:, :, 1, j]
        w[C:2 * C, :, j] = wt[:, :, 0, j]
        w[0:C, :, 3 + j] = wt[:, :, 2, j]
    return np.ascontiguousarray(w.reshape(128, KK * NSLOT * F))


def kernel(x, Wk, bk, att_w1, att_b1, att_w2, att_b2):
    from concourse import bass_utils

    nc = _get_program()

    x2 = _host_pack_x(np.asarray(x))
    wk_h = _host_pack_wk(np.asarray(Wk))
    w1_h = np.ascontiguousarray((att_w1 / (H * W)).astype(np.float32))
    b1_h = np.ascontiguousarray(att_b1.reshape(HID, 1).astype(np.float32))
    w2_h = np.ascontiguousarray((att_w2 / TEMP).astype(np.float32))
    b2_h = np.ascontiguousarray((att_b2 / TEMP).reshape(1, KK)
                                .astype(np.float32))
    bkt = np.transpose(bk, (1, 0)).astype(np.float32)      # [F, K]
    bkt_h = np.ascontiguousarray(np.concatenate([bkt, bkt], axis=0))

    in_maps = []
    for c in range(NCORES):
        in_maps.append({
            "x2": x2[c * BPC:(c + 1) * BPC],
            "wk": wk_h, "w1": w1_h, "b1": b1_h,
            "w2": w2_h, "b2": b2_h, "bkt": bkt_h,
        })

    res = bass_utils.run_bass_kernel_spmd(nc, in_maps,
                                          core_ids=list(range(NCORES)))

    y = np.empty((B, H, W, F), dtype=np.float32)
    for c in range(NCORES):
        yp = res.results[c]["ypad"]                 # [BPC, 128, HALF]
        arr = yp.reshape(BPC, 2, F, H // 2, WP)     # (b, half, f, row, col)
        y[c * BPC:(c + 1) * BPC] = (
            arr[:, :, :, :, 1:W + 1]
            .transpose(0, 1, 3, 4, 2)
            .reshape(BPC, H, W, F)
            .astype(np.float32))
    return y


# revision 15
# speedup vs baseline: 5.4849x; 5.4849x over previous
"""DynamicConv2D Trainium2 kernel (8-core SPMD, data-parallel over batch).

Per sample: GAP -> MLP -> softmax routing over K=4 kernel banks, weight-space
aggregation, then a 3x3 SAME conv with the per-sample aggregated kernel.

Device strategy (per core, 4 samples, software-pipelined across samples):
  - Host packs x TRANSPOSED, width-padded, channel-duplicated bf16
    [128, SP] per sample (rows 0:64 = channels at spatial s, rows 64:128 =
    same shifted one padded image row, so row 64+c at col s holds
    x[c, s-WP]). Plain chunked DMAs load it -- no xbar transposes.
  - Engine/queue assignment keeps every FIFO conflict-free:
      SP    : input chunk loads only
      ACT   : PSUM drains (+bias) and the output store only
      DVE   : pooled partials + the whole routing chain (softmax via a
              3rd-order Taylor exp -- |logits/T| < 1e-3 -- so no ACT hop)
      GpSimd: pad memsets, one pooled partial, pi partition-broadcast
      PE    : routing MLP matmuls + conv matmuls
  - Software pipelining: per iteration emit loads+partials for sample b,
    conv second half of b-1, routing chain of b, conv first half of b.
    The tiny MLP matmuls land mid-stream in the PE queue so the PE never
    idles long enough to drop out of HAM warm state.
  - Conv as shifted matmuls accumulating in PSUM: out[f, p] tiles, f on
    partitions; K=128 paired taps (dy=0 with dy=-1) + K=64 single taps
    (dy=+1); col groups 0/64 of the PE run image halves A/B concurrently.
  - One plain DMA stores yT [f | spatial] per sample; host un-transposes,
    strips width pads, upcasts to fp32.
"""

import numpy as np
import ml_dtypes

BF16 = ml_dtypes.bfloat16

B, H, W, C, F = 32, 128, 128, 64, 64
KK, HID = 4, 16
TEMP = 30.0
NCORES, BPC = 8, 4
WP = W + 2          # padded width (zero col at w'=0 and w'=129)
SP = H * WP         # 16640 padded spatial per sample
PADL = 384          # SBUF zero halo before the image
PADR = 384          # SBUF zero halo after (taps read up to +WP+1)
NTMAX = 416         # matmul moving-dim tile (PSUM bank: <=512 fp32)
HALF = SP // 2      # 8320, image halves A (h<64) / B (h>=64)
TILES = [(i * NTMAX, NTMAX) for i in range(HALF // NTMAX)]
if HALF % NTMAX:
    TILES.append((HALF - HALF % NTMAX, HALF % NTMAX))
TPH = len(TILES)    # 20 tiles per half
NSLOT = 6           # 3 paired-tap slots (K=128) + 3 single-tap slots (K=64)
CHW = SP // 4       # input DMA chunk width (4160 cols, ~1.06 MB)

_CACHE = {}


def _build_program(dbg=False, reps=1):
    import concourse.bacc as bacc
    import concourse.mybir as mybir
    import concourse.tile as tile

    f32 = mybir.dt.float32
    bf16 = mybir.dt.bfloat16
    AX = mybir.AxisListType.X
    ALU = mybir.AluOpType

    nc = bacc.Bacc("TRN2", target_bir_lowering=False, debug=False)

    x2_d = nc.dram_tensor("x2", [BPC, 128, SP], bf16, kind="ExternalInput")
    wk_d = nc.dram_tensor("wk", [128, KK * NSLOT * F], f32,
                          kind="ExternalInput")
    w1_d = nc.dram_tensor("w1", [C, HID], f32, kind="ExternalInput")
    b1_d = nc.dram_tensor("b1", [HID, 1], f32, kind="ExternalInput")
    w2_d = nc.dram_tensor("w2", [HID, KK], f32, kind="ExternalInput")
    b2_d = nc.dram_tensor("b2", [1, KK], f32, kind="ExternalInput")
    bkt_d = nc.dram_tensor("bkt", [128, KK], f32, kind="ExternalInput")
    yp_d = nc.dram_tensor("ypad", [BPC, 128, HALF], bf16,
                          kind="ExternalOutput")
    if dbg:
        dpool_d = nc.dram_tensor("dpool", [BPC, C, 1], f32,
                                 kind="ExternalOutput")
        dpib_d = nc.dram_tensor("dpib", [BPC, 128, KK], f32,
                                kind="ExternalOutput")
        dwg_d = nc.dram_tensor("dwg", [BPC, 128, NSLOT * F], bf16,
                               kind="ExternalOutput")

    with tile.TileContext(nc) as tc:
        from contextlib import ExitStack
        with ExitStack() as ctx:
            cst = ctx.enter_context(tc.tile_pool(name="cst", bufs=1))
            xtp = ctx.enter_context(tc.tile_pool(name="xtp", bufs=3))
            ytp = ctx.enter_context(tc.tile_pool(name="ytp", bufs=2))
            wgp = ctx.enter_context(tc.tile_pool(name="wgp", bufs=2))
            smp = ctx.enter_context(tc.tile_pool(name="smp", bufs=2))
            psp = ctx.enter_context(tc.tile_pool(name="psp", bufs=6,
                                                 space="PSUM"))
            psr = ctx.enter_context(tc.tile_pool(name="psr", bufs=1,
                                                 space="PSUM"))

            # ---- constants ----
            wk_t = cst.tile([128, KK * NSLOT * F], f32)
            nc.sync.dma_start(wk_t[:], wk_d.ap())
            w1_t = cst.tile([C, HID], f32)
            nc.sync.dma_start(w1_t[:], w1_d.ap())
            b1_t = cst.tile([HID, 1], f32)
            nc.sync.dma_start(b1_t[:], b1_d.ap())
            w2_t = cst.tile([HID, KK], f32)
            nc.sync.dma_start(w2_t[:], w2_d.ap())
            b2_t = cst.tile([1, KK], f32)
            nc.sync.dma_start(b2_t[:], b2_d.ap())
            bkt_t = cst.tile([128, KK], f32)
            nc.sync.dma_start(bkt_t[:], bkt_d.ap())
            bagg_t = cst.tile([128, BPC], f32)

            def emit_load(b):
                """Input chunk DMAs + pooled partial reductions."""
                xt = xtp.tile([128, PADL + SP + PADR], bf16, tag="xt")
                nc.gpsimd.memset(xt[:, 0:PADL], 0.0)
                nc.gpsimd.memset(xt[:, PADL + SP:PADL + SP + PADR], 0.0)
                for i in range(4):
                    o = i * CHW
                    nc.sync.dma_start(xt[:, PADL + o:PADL + o + CHW],
                                      x2_d.ap()[b][:, o:o + CHW])
                pp = smp.tile([C, 4], f32, tag="pp")
                for i in range(4):
                    nc.vector.reduce_sum(
                        pp[:, i:i + 1],
                        xt[0:C, PADL + i * CHW:PADL + (i + 1) * CHW],
                        axis=AX)
                return xt, pp

            def emit_chain(b, pp):
                """Routing: pooled -> MLP -> unnormalized softmax -> W_agg.

                The softmax stays UNNORMALIZED (wg = sum_k ex_k Wk); the
                1/sum factor rides into the PSUM drains as a per-partition
                activation scale, so sum/reciprocal leave the critical path.
                """
                pooled = smp.tile([C, 1], f32, tag="pooled")
                nc.vector.reduce_sum(pooled[:], pp[:], axis=AX)
                hps = psr.tile([HID, 1], f32, tag="hps")
                nc.tensor.matmul(hps[:], lhsT=w1_t[:], rhs=pooled[:],
                                 start=True, stop=True)
                h_t = smp.tile([HID, 1], f32, tag="h")
                nc.vector.tensor_scalar(h_t[:], hps[:],
                                        scalar1=b1_t[:, 0:1], scalar2=0.0,
                                        op0=ALU.add, op1=ALU.max)
                lps = psr.tile([1, KK], f32, tag="lps")
                nc.tensor.matmul(lps[:], lhsT=h_t[:], rhs=w2_t[:],
                                 start=True, stop=True)
                lg = smp.tile([1, KK], f32, tag="lg")
                nc.vector.tensor_tensor(lg[:], lps[:], b2_t[:], op=ALU.add)
                # exp(z) ~= 1 + z + z^2/2   (|z| < 1e-3 here)
                e1 = smp.tile([1, KK], f32, tag="e1")
                nc.vector.tensor_scalar(e1[:], lg[:], scalar1=0.5,
                                        scalar2=1.0, op0=ALU.mult,
                                        op1=ALU.add)
                nc.vector.tensor_tensor(e1[:], e1[:], lg[:], op=ALU.mult)
                ex = smp.tile([1, KK], f32, tag="ex")
                nc.vector.tensor_scalar(ex[:], e1[:], scalar1=1.0,
                                        scalar2=None, op0=ALU.add)
                pib = smp.tile([128, KK], f32, tag="pib")
                nc.gpsimd.partition_broadcast(pib[:], ex[:])

                # W_agg = sum_k ex_k * Wk (last FMA writes bf16 directly)
                SF = NSLOT * F
                acc = wgp.tile([128, SF], f32, tag="acc")
                nc.vector.tensor_scalar(acc[:], wk_t[:, 0:SF],
                                        scalar1=pib[:, 0:1], scalar2=None,
                                        op0=ALU.mult)
                for k in range(1, KK - 1):
                    nc.vector.scalar_tensor_tensor(
                        acc[:], wk_t[:, k * SF:(k + 1) * SF],
                        pib[:, k:k + 1], acc[:], op0=ALU.mult, op1=ALU.add)
                wg = wgp.tile([128, SF], bf16, tag="wg")
                nc.vector.scalar_tensor_tensor(
                    wg[:], wk_t[:, (KK - 1) * SF:KK * SF],
                    pib[:, KK - 1:KK], acc[:], op0=ALU.mult, op1=ALU.add)

                # off critical path: 1/sum(ex) for the drains + bias column
                sm = smp.tile([1, 1], f32, tag="sm")
                nc.vector.reduce_sum(sm[:], ex[:], axis=AX)
                rc = smp.tile([1, 1], f32, tag="rc")
                nc.vector.reciprocal(rc[:], sm[:])
                rcb = smp.tile([128, 1], f32, tag="rcb")
                nc.gpsimd.partition_broadcast(rcb[:], rc[:])
                # bagg[:, b] = (sum_k bkT[:, k] * ex_k) * rc
                bu = smp.tile([128, 1], f32, tag="bu")
                nc.vector.tensor_scalar(bu[:], bkt_t[:, 0:1],
                                        scalar1=pib[:, 0:1], scalar2=None,
                                        op0=ALU.mult)
                for k in range(1, KK):
                    nc.vector.scalar_tensor_tensor(
                        bu[:], bkt_t[:, k:k + 1],
                        pib[:, k:k + 1], bu[:],
                        op0=ALU.mult, op1=ALU.add)
                nc.vector.tensor_scalar(bagg_t[:, b:b + 1], bu[:],
                                        scalar1=rcb[:, 0:1], scalar2=None,
                                        op0=ALU.mult)
                if dbg:
                    nc.sync.dma_start(dpool_d.ap()[b], pooled[:])
                    nc.sync.dma_start(dpib_d.ap()[b], pib[:])
                    nc.sync.dma_start(dwg_d.ap()[b], wg[:])
                return wg, rcb

            def emit_conv(b, xt, wg, rcb, yt, t0, t1):
                for t in range(t0, t1):
                    o, w = TILES[t]
                    ps = psp.tile([128, NTMAX], f32, tag="ps")
                    oA = PADL + o
                    oB = oA + HALF
                    for j in range(3):       # taps (0,dx)+(-1,dx), K=128
                        off = j - 1
                        nc.tensor.matmul(
                            ps[0:64, 0:w], lhsT=wg[:, j * F:(j + 1) * F],
                            rhs=xt[:, oA + off:oA + off + w],
                            start=(j == 0), stop=False)
                        nc.tensor.matmul(
                            ps[64:128, 0:w], lhsT=wg[:, j * F:(j + 1) * F],
                            rhs=xt[:, oB + off:oB + off + w],
                            start=(j == 0), stop=False,
                            tile_position=(0, 64))
                    for j in range(3, 6):    # taps (+1,dx), K=64
                        off = WP + (j - 4)
                        nc.tensor.matmul(
                            ps[0:64, 0:w],
                            lhsT=wg[0:64, j * F:(j + 1) * F],
                            rhs=xt[0:64, oA + off:oA + off + w],
                            start=False, stop=(j == 5))
                        nc.tensor.matmul(
                            ps[64:128, 0:w],
                            lhsT=wg[0:64, j * F:(j + 1) * F],
                            rhs=xt[0:64, oB + off:oB + off + w],
                            start=False, stop=(j == 5),
                            tile_position=(0, 64))
                    nc.scalar.activation(
                        yt[:, o:o + w], ps[:, 0:w],
                        mybir.ActivationFunctionType.Identity,
                        bias=bagg_t[:, b:b + 1], scale=rcb[:, 0:1])

            def emit_store(b, yt):
                nc.scalar.dma_start(yp_d.ap()[b], yt[:])

            # ---- software-pipelined main loop: the routing chain for
            # sample s+1 runs during conv of sample s; loads run two ahead.
            S = reps * BPC
            ld = {}
            ch = {}
            ld[0] = emit_load(0)
            ch[0] = emit_chain(0, ld[0][1])
            if S > 1:
                ld[1] = emit_load(1)
            for s in range(S):
                if s + 1 < S:
                    ch[s + 1] = emit_chain((s + 1) % BPC, ld[s + 1][1])
                if s + 2 < S:
                    ld[s + 2] = emit_load((s + 2) % BPC)
                xt, _ = ld.pop(s)
                wg, rcb = ch.pop(s)
                yt = ytp.tile([128, HALF], bf16, tag="yt")
                emit_conv(s % BPC, xt, wg, rcb, yt, 0, TPH)
                emit_store(s % BPC, yt)

    nc.compile()
    return nc


def _get_program():
    if "nc" not in _CACHE:
        _CACHE["nc"] = _build_program()
    return _CACHE["nc"]


def _host_pack_x(x):
    # [B, H, W, C] fp32 -> [B, 128, SP] bf16: rows 0:64 = width-padded x
    # transposed to [c, spatial], rows 64:128 = same, shifted one padded
    # image row (row 64+c at col s holds x[c, s-WP]).
    xb = x.astype(BF16)
    xp = np.zeros((B, H, WP, C), dtype=BF16)
    xp[:, :, 1:W + 1, :] = xb
    flat = xp.reshape(B, SP, C)
    xT = np.ascontiguousarray(flat.transpose(0, 2, 1))   # [B, C, SP]
    x2 = np.zeros((B, 128, SP), dtype=BF16)
    x2[:, 0:C, :] = xT
    x2[:, C:2 * C, WP:] = xT[:, :, 0:SP - WP]
    return x2


def _host_pack_wk(Wk):
    # [K, 3, 3, C, F] -> [128, K*NSLOT*F] fp32. Slot j in 0..2 pairs taps
    # (kh=1, kw=j) on partitions 0:64 with (kh=0, kw=j) on 64:128 (the
    # bottom x half holds the row above). Slots 3/5 hold taps (kh=2,
    # kw=0/2) on partitions 0:64; slot 4 holds tap (kh=2, kw=1) on
    # partitions 64:128 (it reads the shifted copy at offset +2*WP and
    # row-tiles concurrently with slot 3).
    w = np.zeros((128, KK, NSLOT, F), dtype=np.float32)
    wt = np.transpose(Wk, (3, 0, 1, 2, 4))          # [C, K, kh, kw, F]
    for j in range(3):
        w[0:C, :, j] = wt[:, :, 1, j]
        w[C:2 * C, :, j] = wt[:, :, 0, j]
        w[0:C, :, 3 + j] = wt[:, :, 2, j]
    return np.ascontiguousarray(w.reshape(128, KK * NSLOT * F))


def kernel(x, Wk, bk, att_w1, att_b1, att_w2, att_b2):
    from concourse import bass_utils

    nc = _get_program()

    x2 = _host_pack_x(np.asarray(x))
    wk_h = _host_pack_wk(np.asarray(Wk))
    w1_h = np.ascontiguousarray((att_w1 / (H * W)).astype(np.float32))
    b1_h = np.ascontiguousarray(att_b1.reshape(HID, 1).astype(np.float32))
    w2_h = np.ascontiguousarray((att_w2 / TEMP).astype(np.float32))
    b2_h = np.ascontiguousarray((att_b2 / TEMP).reshape(1, KK)
                                .astype(np.float32))
    bkt = np.transpose(bk, (1, 0)).astype(np.float32)      # [F, K]
    bkt_h = np.ascontiguousarray(np.concatenate([bkt, bkt], axis=0))

    in_maps = []
    for c in range(NCORES):
        in_maps.append({
            "x2": x2[c * BPC:(c + 1) * BPC],
            "wk": wk_h, "w1": w1_h, "b1": b1_h,
            "w2": w2_h, "b2": b2_h, "bkt": bkt_h,
        })

    res = bass_utils.run_bass_kernel_spmd(nc, in_maps,
                                          core_ids=list(range(NCORES)))

    y = np.empty((B, H, W, F), dtype=np.float32)
    for c in range(NCORES):
        yp = res.results[c]["ypad"]                 # [BPC, 128, HALF]
        arr = yp.reshape(BPC, 2, F, H // 2, WP)     # (b, half, f, row, col)
        y[c * BPC:(c + 1) * BPC] = (
            arr[:, :, :, :, 1:W + 1]
            .transpose(0, 1, 3, 4, 2)
            .reshape(BPC, H, W, F)
            .astype(np.float32))
    return y
